# revision 13
# baseline (speedup 1.0000x reference)
"""KStoNet (RBF-SVR heads + MLP) fused Trainium2 kernel, data-parallel over 8 cores.

v2: pair-structured pipeline
  - stage-1 x.c matmuls: contraction 64 (the -gamma|x|^2 term is factored out of
    the exp and applied after stage 2), 2-way PE row tiling: two chunks per pair
    computed concurrently on rows 0-63 / 64-127 of the PE array.
  - exp split across ScalarE (true Exp) and VectorE (Schraudolph int16-as-bf16
    fast exp) so both engines stream the 26M elements/core in parallel.
  - stage-2 head reduction: 2-way PE column tiling (64-head groups) into one
    accumulating PSUM bank per 128-head half.
"""
import sys

sys.path.insert(0, "/opt/trn_rl_repo")

import contextlib
import ctypes
import math
import types

import numpy as np


def _install_axon_shims():
    """(1) NTFF profile hook this image's antenv lacks; (2) split the final SP
    Drain's sem waits (this walrus build allows only one sync wait there)."""
    if "antenv.axon_hooks" not in sys.modules:
        lib = ctypes.CDLL("/opt/axon/libaxon_pjrt.so")
        hook = None
        if hasattr(lib, "axon_start_nrt_profile"):
            lib.axon_start_nrt_profile.argtypes = [
                ctypes.POINTER(ctypes.c_int64),
                ctypes.c_size_t,
            ]
            lib.axon_start_nrt_profile.restype = ctypes.c_int64
            lib.axon_stop_nrt_profile.argtypes = [ctypes.c_char_p]
            lib.axon_stop_nrt_profile.restype = ctypes.c_int64

            @contextlib.contextmanager
            def _hook(output_dir, device_ids=None):
                import jax

                jax.devices()
                if device_ids:
                    ids = (ctypes.c_int64 * len(device_ids))(*device_ids)
                    rc = lib.axon_start_nrt_profile(ids, len(device_ids))
                else:
                    rc = lib.axon_start_nrt_profile(None, 0)
                if rc != 0:
                    raise RuntimeError(f"axon_start_nrt_profile rc={rc}")
                try:
                    yield
                finally:
                    n = lib.axon_stop_nrt_profile(str(output_dir).encode())
                    print(f"profile: {n} file(s) -> {output_dir}", file=sys.stderr)

            hook = _hook
        mod = types.ModuleType("antenv.axon_hooks")
        mod.get_axon_ntff_profile_hook = lambda: hook
        mod.set_axon_ntff_profile_hook = lambda h: None
        sys.modules["antenv.axon_hooks"] = mod
        import antenv

        antenv.axon_hooks = mod

    import bass_rust
    import concourse.tile as tile
    from concourse.vector_clock import ScopedClock

    if not getattr(tile.TileContext._drain_and_barrier, "_wait_split", False):

        def _drain_and_barrier(self, tick_clock, wait_clock):
            drain_inst = self.nc.sync.drain()
            wait_clock.add_sem_waits(
                drain_inst.ins, ScopedClock({None: tick_clock.global_clock})
            )
            si = drain_inst.ins.sync_info
            waits = list(si.on_wait) if si and si.on_wait else []
            if len(waits) > 1:
                si.on_wait = waits[:1]
                for w in waits[1:]:
                    extra = self.nc.sync.drain()
                    extra.ins.sync_info = bass_rust.SyncInfo(on_wait=[w], on_update=[])
            self.nc.all_engine_barrier()
            assert self.sems is not None
            popped = self.nc._tile_sem_poison_stack.pop()
            assert popped is self._sem_poison
            self.nc.clear_and_free_semaphores(list(self.sems.allocated().values()))
            self.nc.all_engine_barrier()

        _drain_and_barrier._wait_split = True
        tile.TileContext._drain_and_barrier = _drain_and_barrier


_install_axon_shims()

import ml_dtypes
import concourse.bass as bass
import concourse.tile as tile
from concourse import bacc, mybir
from concourse.bass_utils import run_bass_kernel_spmd

GAMMA = 0.1
B, D, H0, K = 16384, 64, 256, 50
HK = H0 * K  # 12800
NCORES = 8
BC = B // NCORES  # 2048 batch rows per core
SLAB = 512
NSLAB = BC // SLAB  # 4
NPAIR = 50  # chunk pairs: (half h in {0,1}) x (i in 0..24); pair p = 25*h + i
# Schraudolph fast-exp constants (int16 bits of bf16): i = round(A*t + Bc)
SCH_A = 128.0 / math.log(2.0)
SCH_B = 127.0 * 128.0 - 128.0 * 0.0434609
BF16 = mybir.dt.bfloat16
F32 = mybir.dt.float32
I16 = mybir.dt.int16

import os

# exp-engine split: pairs with (i % _DVE_MOD) >= _DVE_PHASE go to the DVE
# Schraudolph path; others to ScalarE Exp. _DVE_MOD=0 disables DVE entirely.
_DVE_MOD = int(os.environ.get("KS_DVE_MOD", "2"))
_DVE_PHASE = int(os.environ.get("KS_DVE_PHASE", "1"))

_CACHE = {}


def _build_program():
    nc = bacc.Bacc("TRN2", target_bir_lowering=False, debug=False)
    x2_d = nc.dram_tensor("x2", [128, BC], BF16, kind="ExternalInput")
    ex2_d = nc.dram_tensor("ex2", [1, BC], BF16, kind="ExternalInput")
    zeros_d = nc.dram_tensor("zeros1", [1, 128], BF16, kind="ExternalInput")
    caug_d = nc.dram_tensor("caug2", [128, NPAIR * 128], BF16, kind="ExternalInput")
    wm_d = nc.dram_tensor("wm2", [128, NPAIR * 128], BF16, kind="ExternalInput")
    wm0_d = nc.dram_tensor("wm0", [128, 256], BF16, kind="ExternalInput")
    svrb_d = nc.dram_tensor("svrb", [128, 2], F32, kind="ExternalInput")
    fcb_d = nc.dram_tensor("fcb", [128, 2], F32, kind="ExternalInput")
    fcT_d = nc.dram_tensor("fcT", [H0, H0], BF16, kind="ExternalInput")
    owT_d = nc.dram_tensor("owT", [H0, 1], BF16, kind="ExternalInput")
    out_d = nc.dram_tensor("out", [BC], F32, kind="ExternalOutput")

    Exp = mybir.ActivationFunctionType.Exp
    Tanh = mybir.ActivationFunctionType.Tanh
    MULT = mybir.AluOpType.mult
    ADD = mybir.AluOpType.add

    with tile.TileContext(nc) as tc:
        with (
            tc.tile_pool(name="const", bufs=1) as constp,
            tc.tile_pool(name="cw", bufs=1) as cwp,
            tc.tile_pool(name="rbf", bufs=4) as rbp,
            tc.tile_pool(name="hid", bufs=2) as hidp,
            tc.tile_pool(name="bcast", bufs=2) as bcastp,
            tc.tile_pool(name="orow", bufs=2) as orowp,
            tc.tile_pool(name="pt1", bufs=2, space=bass.MemorySpace.PSUM) as pt1p,
            tc.tile_pool(name="pacc", bufs=2, space=bass.MemorySpace.PSUM) as paccp,
            tc.tile_pool(name="p34", bufs=2, space=bass.MemorySpace.PSUM) as p34p,
        ):
            # ---- constant loads ----
            x2_sb = constp.tile([128, BC], BF16, tag="x2")
            nc.sync.dma_start(x2_sb[:], x2_d.ap())
            ex2_sb = constp.tile([1, BC], BF16, tag="ex2")
            nc.sync.dma_start(ex2_sb[:], ex2_d.ap())
            zeros_sb = constp.tile([1, 128], BF16, tag="zeros")
            nc.sync.dma_start(zeros_sb[:], zeros_d.ap())
            wm0_sb = constp.tile([128, 256], BF16, tag="wm0")
            nc.sync.dma_start(wm0_sb[:], wm0_d.ap())
            svrb_sb = constp.tile([128, 2], F32, tag="svrb")
            nc.sync.dma_start(svrb_sb[:], svrb_d.ap())
            fcb_sb = constp.tile([128, 2], F32, tag="fcb")
            nc.sync.dma_start(fcb_sb[:], fcb_d.ap())
            fcT_sb = []
            for hh in range(2):
                ft = constp.tile([128, H0], BF16, tag=f"fcT{hh}")
                nc.sync.dma_start(ft[:], fcT_d.ap()[hh * 128 : (hh + 1) * 128, :])
                fcT_sb.append(ft)
            owT_sb = []
            for hh in range(2):
                ot = constp.tile([128, 1], BF16, tag=f"owT{hh}")
                nc.sync.dma_start(ot[:], owT_d.ap()[hh * 128 : (hh + 1) * 128, :])
                owT_sb.append(ot)
            # caug2/wm2 in pieces (over pair-columns) so pair 0 starts ASAP
            PIECES = [2, 3, 4, 6, 10, 12, 13]  # pairs per piece, sums to 50
            piece_of = []
            for ip, npc in enumerate(PIECES):
                for j in range(npc):
                    piece_of.append((ip, j))
            caug_sb = []
            wm_sb = []
            poff = 0
            for ip, npc in enumerate(PIECES):
                ct = cwp.tile([128, npc * 128], BF16, tag=f"caug{ip}", name=f"caug{ip}")
                nc.sync.dma_start(
                    ct[:], caug_d.ap()[:, poff * 128 : (poff + npc) * 128]
                )
                caug_sb.append(ct)
                wt = cwp.tile([128, npc * 128], BF16, tag=f"wm{ip}", name=f"wm{ip}")
                nc.sync.dma_start(wt[:], wm_d.ap()[:, poff * 128 : (poff + npc) * 128])
                wm_sb.append(wt)
                poff += npc

            def caug_ap(p, g):
                ip, j = piece_of[p]
                return caug_sb[ip][64 * g : 64 * g + 64, j * 128 : (j + 1) * 128]

            def wm_ap(p, g):
                ip, j = piece_of[p]
                return wm_sb[ip][:, j * 128 + 64 * g : j * 128 + 64 * g + 64]

            # ---- HAM warm-up: ~5us of dense back-to-back matmuls so the PE
            # clock-gate fires K=8/8 before the real pipeline starts (the
            # steady-state stream has micro-gaps and never warms on its own).
            # Overlaps the caug2/wm2 DMA lead-in; only needs the tiny
            # zeros/ex2 DMAs.
            warm = p34p.tile([128, SLAB], F32, tag="p34", name="warm")
            for w in range(12):
                nc.tensor.matmul(
                    warm[:],
                    zeros_sb[0:1, :],
                    ex2_sb[0:1, 0:SLAB],
                    start=True,
                    stop=True,
                    skip_group_check=True,
                )

            # ---- main loop ----
            deferred = []
            hidT_of = {}
            for s in range(NSLAB):
                bsl = slice(s * SLAB, (s + 1) * SLAB)
                # bcast[128, SLAB] = exp(-gamma*|x_b|^2) broadcast across partitions
                bcast = bcastp.tile([128, SLAB], BF16, tag="bcast")
                nc.gpsimd.partition_broadcast(bcast[:], ex2_sb[0:1, bsl])

                for h in range(2):
                    accT = paccp.tile([128, SLAB], F32, tag="acc", name=f"acc{h}")
                    for i in range(25):
                        if i == 2:
                            for f in deferred:
                                f()
                            deferred.clear()
                        p = 25 * h + i
                        pt1 = pt1p.tile([128, 2 * SLAB], F32, tag="pt1")
                        for g in range(2):
                            nc.tensor.matmul(
                                pt1[:, g * SLAB : (g + 1) * SLAB],
                                caug_ap(p, g),
                                x2_sb[64 * g : 64 * g + 64, bsl],
                                start=True,
                                stop=True,
                            )
                        rb = rbp.tile([128, 2 * SLAB], BF16, tag="rb")
                        use_dve = _DVE_MOD > 0 and (i % _DVE_MOD) >= _DVE_PHASE
                        if not use_dve:
                            nc.scalar.activation(rb[:], pt1[:], Exp)
                        else:
                            nc.vector.tensor_scalar(
                                rb[:].bitcast(I16), pt1[:], SCH_A, SCH_B, MULT, ADD
                            )
                        if i == 0:
                            # full-width stationary (g0 weights | zeros): writes the
                            # whole bank with start=True, so it both seeds rows
                            # 64-127 with zeros and WAW-orders itself before the
                            # g1 accumulation chain.
                            nc.tensor.matmul(
                                accT[:],
                                wm0_sb[:, h * 128 : (h + 1) * 128],
                                rb[:, 0:SLAB],
                                start=True,
                                stop=False,
                                skip_group_check=True,
                            )
                            nc.tensor.matmul(
                                accT[64:128, :],
                                wm_ap(p, 1),
                                rb[:, SLAB : 2 * SLAB],
                                start=False,
                                stop=False,
                                skip_group_check=True,
                            )
                        else:
                            for g in range(2):
                                nc.tensor.matmul(
                                    accT[64 * g : 64 * g + 64, :],
                                    wm_ap(p, g),
                                    rb[:, g * SLAB : (g + 1) * SLAB],
                                    start=False,
                                    stop=(i == 24 and g == 1),
                                    skip_group_check=True,
                                )

                    def tail(s=s, h=h, accT=accT, bcast=bcast, bsl=bsl):
                        hpre = hidp.tile([128, SLAB], F32, tag="hpre")
                        nc.vector.tensor_tensor(hpre[:], accT[:], bcast[:], MULT)
                        ht = hidp.tile([128, SLAB], BF16, tag="hidT")
                        nc.scalar.activation(
                            ht[:], hpre[:], Tanh, bias=svrb_sb[:, h : h + 1]
                        )
                        hidT_of[(s, h)] = ht
                        if h == 1:
                            hidT = [hidT_of.pop((s, 0)), hidT_of.pop((s, 1))]
                            h2T = [None, None]
                            for jh in range(2):
                                psB = p34p.tile([128, SLAB], F32, tag="p34", name="psB")
                                for hh in range(2):
                                    nc.tensor.matmul(
                                        psB[:],
                                        fcT_sb[hh][:, jh * 128 : (jh + 1) * 128],
                                        hidT[hh][:],
                                        start=(hh == 0),
                                        stop=(hh == 1),
                                    )
                                h2 = hidp.tile([128, SLAB], BF16, tag="h2T")
                                nc.scalar.activation(
                                    h2[:], psB[:], Tanh, bias=fcb_sb[:, jh : jh + 1]
                                )
                                h2T[jh] = h2
                            psC = p34p.tile([1, SLAB], F32, tag="p34", name="psC")
                            for jh in range(2):
                                nc.tensor.matmul(
                                    psC[:],
                                    owT_sb[jh][:],
                                    h2T[jh][:],
                                    start=(jh == 0),
                                    stop=(jh == 1),
                                )
                            orow = orowp.tile([1, SLAB], F32, tag="orow")
                            nc.vector.tensor_copy(orow[:], psC[:])
                            nc.sync.dma_start(
                                out_d.ap()[s * SLAB : (s + 1) * SLAB], orow[0:1, :]
                            )

                    deferred.append(tail)
            for f in deferred:
                f()
            deferred.clear()
    nc.compile()
    return nc


def _prep_inputs(x, centers, svr_w, svr_b, fc_w, fc_b, out_w, out_b):
    bf16 = ml_dtypes.bfloat16
    x = np.asarray(x, np.float32)
    cfl = np.asarray(centers, np.float32)  # [H0, K, D]
    svr_w = np.asarray(svr_w, np.float32)
    c2 = (cfl * cfl).sum(-1)  # [H0, K]
    wfold = svr_w * np.exp(-GAMMA * c2)  # [H0, K]

    # hk order within a (h, g) 64-head group: j = 128*i + p, head_local=j//50, k=j%50
    j = np.arange(64 * K)
    hl = j // K
    kk = j % K
    caug2 = np.empty((128, NPAIR, 128), np.float32)
    wm2 = np.zeros((128, NPAIR, 128), np.float32)
    for h in range(2):
        for g in range(2):
            heads = 128 * h + 64 * g + hl  # [3200]
            cm = 2.0 * GAMMA * cfl[heads, kk, :]  # [3200, D]
            caug2[64 * g : 64 * g + 64, 25 * h : 25 * h + 25, :] = cm.T.reshape(
                D, 25, 128
            )
            W = np.zeros((64 * K, 64), np.float32)
            W[j, hl] = wfold[heads, kk]
            wm2[:, 25 * h : 25 * h + 25, 64 * g : 64 * g + 64] = W.reshape(
                25, 128, 64
            ).transpose(1, 0, 2)
    caug2 = caug2.reshape(128, NPAIR * 128).astype(bf16)
    wm2 = wm2.reshape(128, NPAIR * 128).astype(bf16)

    x2 = np.empty((128, B), bf16)
    x2[:D] = x.T.astype(bf16)
    x2[D:] = x2[:D]
    ex2 = np.exp(-GAMMA * (x * x).sum(-1)).astype(bf16).reshape(1, B)
    zeros1 = np.zeros((1, 128), bf16)
    wm2v = wm2.reshape(128, NPAIR, 128)
    wm0 = np.zeros((128, 256), bf16)
    for h in range(2):
        wm0[:, h * 128 : h * 128 + 64] = wm2v[:, 25 * h, 0:64]
    wm0 = np.ascontiguousarray(wm0)
    svrb = np.stack(
        [np.asarray(svr_b, np.float32)[:128], np.asarray(svr_b, np.float32)[128:]], 1
    )
    fcb = np.stack(
        [np.asarray(fc_b, np.float32)[:128], np.asarray(fc_b, np.float32)[128:]], 1
    )
    fcT = np.ascontiguousarray(np.asarray(fc_w, np.float32).T.astype(bf16))
    owT = np.ascontiguousarray(np.asarray(out_w, np.float32).T.astype(bf16))
    return x2, ex2, zeros1, wm0, caug2, wm2, svrb, fcb, fcT, owT, float(np.asarray(out_b)[0])


def kernel(x, centers, svr_w, svr_b, fc_w, fc_b, out_w, out_b, _trace=False):
    if "nc" not in _CACHE:
        _CACHE["nc"] = _build_program()
    nc = _CACHE["nc"]
    x2, ex2, zeros1, wm0, caug2, wm2, svrb, fcb, fcT, owT, ob = _prep_inputs(
        x, centers, svr_w, svr_b, fc_w, fc_b, out_w, out_b
    )
    in_maps = []
    for c in range(NCORES):
        in_maps.append(
            {
                "x2": np.ascontiguousarray(x2[:, c * BC : (c + 1) * BC]),
                "ex2": np.ascontiguousarray(ex2[:, c * BC : (c + 1) * BC]),
                "zeros1": zeros1,
                "wm0": wm0,
                "caug2": caug2,
                "wm2": wm2,
                "svrb": svrb,
                "fcb": fcb,
                "fcT": fcT,
                "owT": owT,
            }
        )
    res = run_bass_kernel_spmd(nc, in_maps, list(range(NCORES)), trace=_trace)
    out = np.concatenate([res.results[c]["out"] for c in range(NCORES)])
    out = (out + ob).astype(np.float32).reshape(B, 1)
    if _trace:
        kernel._last_results = res
    return out


# revision 14
# speedup vs baseline: 1.4053x; 1.4053x over previous
"""KStoNet (RBF-SVR heads + MLP) fused Trainium2 kernel, data-parallel over 8 cores.

v2: pair-structured pipeline
  - stage-1 x.c matmuls: contraction 64 (the -gamma|x|^2 term is factored out of
    the exp and applied after stage 2), 2-way PE row tiling: two chunks per pair
    computed concurrently on rows 0-63 / 64-127 of the PE array.
  - exp split across ScalarE (true Exp) and VectorE (Schraudolph int16-as-bf16
    fast exp) so both engines stream the 26M elements/core in parallel.
  - stage-2 head reduction: 2-way PE column tiling (64-head groups) into one
    accumulating PSUM bank per 128-head half.
"""
import sys

sys.path.insert(0, "/opt/trn_rl_repo")

import contextlib
import ctypes
import math
import types

import numpy as np


def _install_axon_shims():
    """(1) NTFF profile hook this image's antenv lacks; (2) split the final SP
    Drain's sem waits (this walrus build allows only one sync wait there)."""
    if "antenv.axon_hooks" not in sys.modules:
        lib = ctypes.CDLL("/opt/axon/libaxon_pjrt.so")
        hook = None
        if hasattr(lib, "axon_start_nrt_profile"):
            lib.axon_start_nrt_profile.argtypes = [
                ctypes.POINTER(ctypes.c_int64),
                ctypes.c_size_t,
            ]
            lib.axon_start_nrt_profile.restype = ctypes.c_int64
            lib.axon_stop_nrt_profile.argtypes = [ctypes.c_char_p]
            lib.axon_stop_nrt_profile.restype = ctypes.c_int64

            @contextlib.contextmanager
            def _hook(output_dir, device_ids=None):
                import jax

                jax.devices()
                if device_ids:
                    ids = (ctypes.c_int64 * len(device_ids))(*device_ids)
                    rc = lib.axon_start_nrt_profile(ids, len(device_ids))
                else:
                    rc = lib.axon_start_nrt_profile(None, 0)
                if rc != 0:
                    raise RuntimeError(f"axon_start_nrt_profile rc={rc}")
                try:
                    yield
                finally:
                    n = lib.axon_stop_nrt_profile(str(output_dir).encode())
                    print(f"profile: {n} file(s) -> {output_dir}", file=sys.stderr)

            hook = _hook
        mod = types.ModuleType("antenv.axon_hooks")
        mod.get_axon_ntff_profile_hook = lambda: hook
        mod.set_axon_ntff_profile_hook = lambda h: None
        sys.modules["antenv.axon_hooks"] = mod
        import antenv

        antenv.axon_hooks = mod

    import bass_rust
    import concourse.tile as tile
    from concourse.vector_clock import ScopedClock

    if not getattr(tile.TileContext._drain_and_barrier, "_wait_split", False):

        def _drain_and_barrier(self, tick_clock, wait_clock):
            drain_inst = self.nc.sync.drain()
            wait_clock.add_sem_waits(
                drain_inst.ins, ScopedClock({None: tick_clock.global_clock})
            )
            si = drain_inst.ins.sync_info
            waits = list(si.on_wait) if si and si.on_wait else []
            if len(waits) > 1:
                si.on_wait = waits[:1]
                for w in waits[1:]:
                    extra = self.nc.sync.drain()
                    extra.ins.sync_info = bass_rust.SyncInfo(on_wait=[w], on_update=[])
            self.nc.all_engine_barrier()
            assert self.sems is not None
            popped = self.nc._tile_sem_poison_stack.pop()
            assert popped is self._sem_poison
            self.nc.clear_and_free_semaphores(list(self.sems.allocated().values()))
            self.nc.all_engine_barrier()

        _drain_and_barrier._wait_split = True
        tile.TileContext._drain_and_barrier = _drain_and_barrier


_install_axon_shims()

import ml_dtypes
import concourse.bass as bass
import concourse.tile as tile
from concourse import bacc, mybir
from concourse.bass_utils import run_bass_kernel_spmd

GAMMA = 0.1
B, D, H0, K = 16384, 64, 256, 50
HK = H0 * K  # 12800
NCORES = 8
BC = B // NCORES  # 2048 batch rows per core
SLAB = 512
NSLAB = BC // SLAB  # 4
NPAIR = 50  # chunk pairs: (half h in {0,1}) x (i in 0..24); pair p = 25*h + i
# Schraudolph fast-exp constants (int16 bits of bf16): i = round(A*t + Bc)
SCH_A = 128.0 / math.log(2.0)
SCH_B = 127.0 * 128.0 - 128.0 * 0.0434609
BF16 = mybir.dt.bfloat16
F32 = mybir.dt.float32
I16 = mybir.dt.int16

import os

# exp-engine split: pairs with (i % _DVE_MOD) >= _DVE_PHASE go to the DVE
# Schraudolph path; others to ScalarE Exp. _DVE_MOD=0 disables DVE entirely.
_DVE_MOD = int(os.environ.get("KS_DVE_MOD", "2"))
_DVE_PHASE = int(os.environ.get("KS_DVE_PHASE", "1"))

_CACHE = {}


def _build_program():
    nc = bacc.Bacc("TRN2", target_bir_lowering=False, debug=False)
    x2_d = nc.dram_tensor("x2", [128, BC], BF16, kind="ExternalInput")
    ex2_d = nc.dram_tensor("ex2", [1, BC], BF16, kind="ExternalInput")
    zeros_d = nc.dram_tensor("zeros1", [1, 128], BF16, kind="ExternalInput")
    caug_d = nc.dram_tensor("caug2", [128, NPAIR * 128], BF16, kind="ExternalInput")
    wm_d = nc.dram_tensor("wm2", [128, NPAIR * 128], BF16, kind="ExternalInput")
    wm0_d = nc.dram_tensor("wm0", [128, 256], BF16, kind="ExternalInput")
    svrb_d = nc.dram_tensor("svrb", [128, 2], F32, kind="ExternalInput")
    fcb_d = nc.dram_tensor("fcb", [128, 2], F32, kind="ExternalInput")
    fcT_d = nc.dram_tensor("fcT", [H0, H0], BF16, kind="ExternalInput")
    owT_d = nc.dram_tensor("owT", [H0, 1], BF16, kind="ExternalInput")
    out_d = nc.dram_tensor("out", [BC], F32, kind="ExternalOutput")

    Exp = mybir.ActivationFunctionType.Exp
    Tanh = mybir.ActivationFunctionType.Tanh
    MULT = mybir.AluOpType.mult
    ADD = mybir.AluOpType.add

    with tile.TileContext(nc) as tc:
        with (
            tc.tile_pool(name="const", bufs=1) as constp,
            tc.tile_pool(name="cw", bufs=1) as cwp,
            tc.tile_pool(name="rbf", bufs=4) as rbp,
            tc.tile_pool(name="hid", bufs=2) as hidp,
            tc.tile_pool(name="bcast", bufs=2) as bcastp,
            tc.tile_pool(name="orow", bufs=2) as orowp,
            tc.tile_pool(name="pt1", bufs=3, space=bass.MemorySpace.PSUM) as pt1p,
            tc.tile_pool(name="pacc", bufs=2, space=bass.MemorySpace.PSUM) as paccp,
        ):
            # ---- constant loads ----
            x2_sb = constp.tile([128, BC], BF16, tag="x2")
            nc.sync.dma_start(x2_sb[:], x2_d.ap())
            ex2_sb = constp.tile([1, BC], BF16, tag="ex2")
            nc.sync.dma_start(ex2_sb[:], ex2_d.ap())
            zeros_sb = constp.tile([1, 128], BF16, tag="zeros")
            nc.sync.dma_start(zeros_sb[:], zeros_d.ap())
            wm0_sb = constp.tile([128, 256], BF16, tag="wm0")
            nc.sync.dma_start(wm0_sb[:], wm0_d.ap())
            svrb_sb = constp.tile([128, 2], F32, tag="svrb")
            nc.sync.dma_start(svrb_sb[:], svrb_d.ap())
            fcb_sb = constp.tile([128, 2], F32, tag="fcb")
            nc.sync.dma_start(fcb_sb[:], fcb_d.ap())
            fcT_sb = []
            for hh in range(2):
                ft = constp.tile([128, H0], BF16, tag=f"fcT{hh}")
                nc.sync.dma_start(ft[:], fcT_d.ap()[hh * 128 : (hh + 1) * 128, :])
                fcT_sb.append(ft)
            owT_sb = []
            for hh in range(2):
                ot = constp.tile([128, 1], BF16, tag=f"owT{hh}")
                nc.sync.dma_start(ot[:], owT_d.ap()[hh * 128 : (hh + 1) * 128, :])
                owT_sb.append(ot)
            # caug2/wm2 in pieces (over pair-columns) so pair 0 starts ASAP
            PIECES = [2, 3, 4, 6, 10, 12, 13]  # pairs per piece, sums to 50
            piece_of = []
            for ip, npc in enumerate(PIECES):
                for j in range(npc):
                    piece_of.append((ip, j))
            caug_sb = []
            wm_sb = []
            poff = 0
            for ip, npc in enumerate(PIECES):
                ct = cwp.tile([128, npc * 128], BF16, tag=f"caug{ip}", name=f"caug{ip}")
                nc.sync.dma_start(
                    ct[:], caug_d.ap()[:, poff * 128 : (poff + npc) * 128]
                )
                caug_sb.append(ct)
                wt = cwp.tile([128, npc * 128], BF16, tag=f"wm{ip}", name=f"wm{ip}")
                nc.sync.dma_start(wt[:], wm_d.ap()[:, poff * 128 : (poff + npc) * 128])
                wm_sb.append(wt)
                poff += npc

            def caug_ap(p, g):
                ip, j = piece_of[p]
                return caug_sb[ip][64 * g : 64 * g + 64, j * 128 : (j + 1) * 128]

            def wm_ap(p, g):
                ip, j = piece_of[p]
                return wm_sb[ip][:, j * 128 + 64 * g : j * 128 + 64 * g + 64]

            # ---- HAM warm-up: ~5us of dense back-to-back matmuls so the PE
            # clock-gate fires K=8/8 before the real pipeline starts (the
            # steady-state stream has micro-gaps and never warms on its own).
            # Overlaps the caug2/wm2 DMA lead-in; only needs the tiny
            # zeros/ex2 DMAs.
            warm = paccp.tile([128, SLAB], F32, tag="acc", name="warm")
            for w in range(12):
                nc.tensor.matmul(
                    warm[:],
                    zeros_sb[0:1, :],
                    ex2_sb[0:1, 0:SLAB],
                    start=True,
                    stop=True,
                    skip_group_check=True,
                )

            # ---- main loop ----
            deferred = []
            hidT_of = {}
            for s in range(NSLAB):
                bsl = slice(s * SLAB, (s + 1) * SLAB)
                # bcast[128, SLAB] = exp(-gamma*|x_b|^2) broadcast across partitions
                bcast = bcastp.tile([128, SLAB], BF16, tag="bcast")
                nc.gpsimd.partition_broadcast(bcast[:], ex2_sb[0:1, bsl])

                for h in range(2):
                    accT = paccp.tile([128, SLAB], F32, tag="acc", name=f"acc{h}")
                    for i in range(25):
                        if i == 2:
                            for f in deferred:
                                f()
                            deferred.clear()
                        p = 25 * h + i
                        pt1 = pt1p.tile([128, 2 * SLAB], F32, tag="pt1")
                        for g in range(2):
                            nc.tensor.matmul(
                                pt1[:, g * SLAB : (g + 1) * SLAB],
                                caug_ap(p, g),
                                x2_sb[64 * g : 64 * g + 64, bsl],
                                start=True,
                                stop=True,
                            )
                        rb = rbp.tile([128, 2 * SLAB], BF16, tag="rb")
                        use_dve = _DVE_MOD > 0 and (i % _DVE_MOD) >= _DVE_PHASE
                        if not use_dve:
                            nc.scalar.activation(rb[:], pt1[:], Exp)
                        else:
                            nc.vector.tensor_scalar(
                                rb[:].bitcast(I16), pt1[:], SCH_A, SCH_B, MULT, ADD
                            )
                        if i == 0:
                            # full-width stationary (g0 weights | zeros): writes the
                            # whole bank with start=True, so it both seeds rows
                            # 64-127 with zeros and WAW-orders itself before the
                            # g1 accumulation chain.
                            nc.tensor.matmul(
                                accT[:],
                                wm0_sb[:, h * 128 : (h + 1) * 128],
                                rb[:, 0:SLAB],
                                start=True,
                                stop=False,
                                skip_group_check=True,
                            )
                            nc.tensor.matmul(
                                accT[64:128, :],
                                wm_ap(p, 1),
                                rb[:, SLAB : 2 * SLAB],
                                start=False,
                                stop=False,
                                skip_group_check=True,
                            )
                        else:
                            for g in range(2):
                                nc.tensor.matmul(
                                    accT[64 * g : 64 * g + 64, :],
                                    wm_ap(p, g),
                                    rb[:, g * SLAB : (g + 1) * SLAB],
                                    start=False,
                                    stop=(i == 24 and g == 1),
                                    skip_group_check=True,
                                )

                    def tail(s=s, h=h, accT=accT, bcast=bcast, bsl=bsl):
                        hpre = hidp.tile([128, SLAB], F32, tag="hpre")
                        nc.vector.tensor_tensor(hpre[:], accT[:], bcast[:], MULT)
                        ht = hidp.tile([128, SLAB], BF16, tag="hidT")
                        nc.scalar.activation(
                            ht[:], hpre[:], Tanh, bias=svrb_sb[:, h : h + 1]
                        )
                        hidT_of[(s, h)] = ht
                        if h == 1:
                            hidT = [hidT_of.pop((s, 0)), hidT_of.pop((s, 1))]
                            h2T = [None, None]
                            for jh in range(2):
                                psB = paccp.tile([128, SLAB], F32, tag="acc", name="psB")
                                for hh in range(2):
                                    nc.tensor.matmul(
                                        psB[:],
                                        fcT_sb[hh][:, jh * 128 : (jh + 1) * 128],
                                        hidT[hh][:],
                                        start=(hh == 0),
                                        stop=(hh == 1),
                                    )
                                h2 = hidp.tile([128, SLAB], BF16, tag="h2T")
                                nc.scalar.activation(
                                    h2[:], psB[:], Tanh, bias=fcb_sb[:, jh : jh + 1]
                                )
                                h2T[jh] = h2
                            psC = paccp.tile([1, SLAB], F32, tag="acc", name="psC")
                            for jh in range(2):
                                nc.tensor.matmul(
                                    psC[:],
                                    owT_sb[jh][:],
                                    h2T[jh][:],
                                    start=(jh == 0),
                                    stop=(jh == 1),
                                )
                            orow = orowp.tile([1, SLAB], F32, tag="orow")
                            nc.vector.tensor_copy(orow[:], psC[:])
                            nc.sync.dma_start(
                                out_d.ap()[s * SLAB : (s + 1) * SLAB], orow[0:1, :]
                            )

                    deferred.append(tail)
            for f in deferred:
                f()
            deferred.clear()
    nc.compile()
    return nc


def _prep_inputs(x, centers, svr_w, svr_b, fc_w, fc_b, out_w, out_b):
    bf16 = ml_dtypes.bfloat16
    x = np.asarray(x, np.float32)
    cfl = np.asarray(centers, np.float32)  # [H0, K, D]
    svr_w = np.asarray(svr_w, np.float32)
    c2 = (cfl * cfl).sum(-1)  # [H0, K]
    wfold = svr_w * np.exp(-GAMMA * c2)  # [H0, K]

    # hk order within a (h, g) 64-head group: j = 128*i + p, head_local=j//50, k=j%50
    j = np.arange(64 * K)
    hl = j // K
    kk = j % K
    caug2 = np.empty((128, NPAIR, 128), np.float32)
    wm2 = np.zeros((128, NPAIR, 128), np.float32)
    for h in range(2):
        for g in range(2):
            heads = 128 * h + 64 * g + hl  # [3200]
            cm = 2.0 * GAMMA * cfl[heads, kk, :]  # [3200, D]
            caug2[64 * g : 64 * g + 64, 25 * h : 25 * h + 25, :] = cm.T.reshape(
                D, 25, 128
            )
            W = np.zeros((64 * K, 64), np.float32)
            W[j, hl] = wfold[heads, kk]
            wm2[:, 25 * h : 25 * h + 25, 64 * g : 64 * g + 64] = W.reshape(
                25, 128, 64
            ).transpose(1, 0, 2)
    caug2 = caug2.reshape(128, NPAIR * 128).astype(bf16)
    wm2 = wm2.reshape(128, NPAIR * 128).astype(bf16)

    x2 = np.empty((128, B), bf16)
    x2[:D] = x.T.astype(bf16)
    x2[D:] = x2[:D]
    ex2 = np.exp(-GAMMA * (x * x).sum(-1)).astype(bf16).reshape(1, B)
    zeros1 = np.zeros((1, 128), bf16)
    wm2v = wm2.reshape(128, NPAIR, 128)
    wm0 = np.zeros((128, 256), bf16)
    for h in range(2):
        wm0[:, h * 128 : h * 128 + 64] = wm2v[:, 25 * h, 0:64]
    wm0 = np.ascontiguousarray(wm0)
    svrb = np.stack(
        [np.asarray(svr_b, np.float32)[:128], np.asarray(svr_b, np.float32)[128:]], 1
    )
    fcb = np.stack(
        [np.asarray(fc_b, np.float32)[:128], np.asarray(fc_b, np.float32)[128:]], 1
    )
    fcT = np.ascontiguousarray(np.asarray(fc_w, np.float32).T.astype(bf16))
    owT = np.ascontiguousarray(np.asarray(out_w, np.float32).T.astype(bf16))
    return x2, ex2, zeros1, wm0, caug2, wm2, svrb, fcb, fcT, owT, float(np.asarray(out_b)[0])


def kernel(x, centers, svr_w, svr_b, fc_w, fc_b, out_w, out_b, _trace=False):
    if "nc" not in _CACHE:
        _CACHE["nc"] = _build_program()
    nc = _CACHE["nc"]
    x2, ex2, zeros1, wm0, caug2, wm2, svrb, fcb, fcT, owT, ob = _prep_inputs(
        x, centers, svr_w, svr_b, fc_w, fc_b, out_w, out_b
    )
    in_maps = []
    for c in range(NCORES):
        in_maps.append(
            {
                "x2": np.ascontiguousarray(x2[:, c * BC : (c + 1) * BC]),
                "ex2": np.ascontiguousarray(ex2[:, c * BC : (c + 1) * BC]),
                "zeros1": zeros1,
                "wm0": wm0,
                "caug2": caug2,
                "wm2": wm2,
                "svrb": svrb,
                "fcb": fcb,
                "fcT": fcT,
                "owT": owT,
            }
        )
    res = run_bass_kernel_spmd(nc, in_maps, list(range(NCORES)), trace=_trace)
    out = np.concatenate([res.results[c]["out"] for c in range(NCORES)])
    out = (out + ob).astype(np.float32).reshape(B, 1)
    if _trace:
        kernel._last_results = res
    return out
